# revision 6
# baseline (speedup 1.0000x reference)
"""Multi-head attention TRN2 Bass kernel (8 NeuronCores).

Problem: B=4, S=2048, D_MODEL=1024, H=16, d_k=d_v=64 (fp32 in/out).

Sharding: core c handles batch b=c//2 and head-half hh=c%2 (8 heads).
Each core computes partial_out = softmax(qh@khT/8) @ vh @ Wo[rows of its
heads]; the host sums the two partials per batch.

The kernel is ACT(exp)-bound: 33.5M exps/core at 1 elem/cycle/lane
@1.2GHz (+284cyc per instr) = 277us ACT busy vs 273us of PE streaming.
The schedule therefore keeps the exp stream saturated end-to-end:

  - pair-major block order: attention blocks run (sq 0..3) x pair 0,
    then pair 1, ... so the projection work for pairs 1-3 (and the V
    projection) spreads into PE slack slots instead of bunching before
    the first exp.
  - prologue projects only khT/qhT (pair 0, sb0) before the first
    scores; everything else is interleaved as per-slot "extras".
  - kT/qT staging tiles are re-DMAed per projection pass (SBUF can't
    hold them across the deferred pair projections).
  - PSUM: scores 2x[128,1024] (double-buffered exp pipeline),
    av 1x[128,1024] (released early via an fp32 copy to SBUF before
    the reciprocal/muls), wo/projection chunks share 1x[128,1024].
  - scores are emitted `lag` skt-slots ahead of the matching AV matmul
    so an exp-wait never head-of-line-blocks the next scores in the
    in-order PE queue.
  - Wo for each sq runs as slack-filler extras in the pair-3 phase;
    only wo(3) trails the last block.
"""

import numpy as np

import concourse.bass as bass  # noqa: F401
import concourse.mybir as mybir
import concourse.tile as tile
from concourse import bacc
from concourse.bass_utils import run_bass_kernel_spmd

S = 2048  # sequence length
D = 1024  # d_model
HPC = 8  # heads per core
DK = 64  # head dim
HD = HPC * DK  # 512: projected width per core
N_CORES = 8

SB = S // 512  # 4 s-blocks of 512
KT = D // 128  # 8 contraction tiles for projections
SKT = S // 128  # 16 key tiles
F32 = mybir.dt.float32
F16 = mybir.dt.float16

_CACHE = {}


def _build():
    nc = bacc.Bacc("TRN2", target_bir_lowering=False, debug=False, num_devices=N_CORES)
    qT = nc.dram_tensor("qT", [D, S], F16, kind="ExternalInput")
    kT = nc.dram_tensor("kT", [D, S], F16, kind="ExternalInput")
    vT = nc.dram_tensor("vT", [D, S], F16, kind="ExternalInput")
    wq = nc.dram_tensor("wq", [D, HD], F16, kind="ExternalInput")
    wk = nc.dram_tensor("wk", [D, HD], F16, kind="ExternalInput")
    wv = nc.dram_tensor("wv", [D, HD], F16, kind="ExternalInput")
    wo = nc.dram_tensor("wo", [HD, D], F16, kind="ExternalInput")
    out = nc.dram_tensor("out", [S, D], F16, kind="ExternalOutput")

    with tile.TileContext(nc) as tc:
        with (
            tc.tile_pool(name="resident", bufs=1) as resident,
            tc.tile_pool(name="stage", bufs=2) as stage,
            tc.tile_pool(name="et", bufs=8) as etp,
            tc.tile_pool(name="misc", bufs=2) as misc,
            tc.tile_pool(name="stk", bufs=16) as stkp,
            tc.tile_pool(name="outst", bufs=2) as outstp,
            tc.tile_pool(name="psum", bufs=2, space="PSUM") as psum,
        ):
            # --- resident tiles ---
            wv16 = resident.tile([128, KT, HD], F16)
            wk16 = resident.tile([128, KT, HD], F16)
            wq16 = resident.tile([128, KT, HD], F16)
            wo16 = resident.tile([128, HD // 128, D], F16)
            qhT = resident.tile([128, HPC // 2, S], F16)  # [2-head tile, pair, Sq]
            khT = resident.tile([128, HPC // 2, S], F16)
            # AV stationary: [..., 0:64] = 1.0 (denominator), [..., 64:128] = vh
            vh = resident.tile([128, SKT, HPC, 128], F16)
            nc.vector.memset(vh[:, :, :, 0:DK], 1.0)

            kre = kT.ap().rearrange("(t p) s -> p t s", p=128)
            qre = qT.ap().rearrange("(t p) s -> p t s", p=128)
            vre = vT.ap().rearrange("(t p) s -> p t s", p=128)

            def dma_w_chunk(w16, wsrc, m, q=0):
                eng = nc.sync if q == 0 else nc.scalar
                eng.dma_start(
                    out=w16[:, :, m * 128 : (m + 1) * 128],
                    in_=wsrc.ap().rearrange("(t p) m -> p t m", p=128)[
                        :, :, m * 128 : (m + 1) * 128
                    ],
                )

            def stage_tile(src_re, sb, tag, q=0):
                st = stage.tile([128, KT, 512], F16, tag=tag, name=f"st_{tag}")
                eng = nc.sync if q == 0 else nc.scalar
                for t in range(KT):
                    eng.dma_start(out=st[:, t, :], in_=src_re[:, t, sb * 512 : (sb + 1) * 512])
                return st

            def proj_chunk(dstT, w16, st, m, sb):
                """One sb-chunk of the m-th pair projection: 8 MMs + copy."""
                ps = psum.tile([128, 1024], F32, tag="wo", bufs=1, name="pps")
                for t in range(KT):
                    nc.tensor.matmul(
                        ps[:, 0:512],
                        lhsT=w16[:, t, m * 128 : (m + 1) * 128],
                        rhs=st[:, t, :],
                        start=(t == 0),
                        stop=(t == KT - 1),
                    )
                nc.vector.tensor_copy(
                    dstT[:, m, sb * 512 : (sb + 1) * 512], ps[:, 0:512]
                )

            def pv_dchunk(vsts, j):
                """V projection for skt tiles 2j, 2j+1 (all 8 heads)."""
                ps = psum.tile([128, 1024], F32, tag="wo", bufs=1, name="pvps")
                for half in range(2):
                    c = 2 * j + half
                    sb, cc = c // 4, c % 4
                    for t in range(KT):
                        nc.tensor.matmul(
                            ps[:, half * 512 : (half + 1) * 512],
                            lhsT=vsts[sb][:, t, cc * 128 : (cc + 1) * 128],
                            rhs=wv16[:, t, :],
                            start=(t == 0),
                            stop=(t == KT - 1),
                        )
                nc.vector.tensor_copy(
                    vh[:, 2 * j : 2 * j + 2, :, DK:128],
                    ps.rearrange("p (s h d) -> p s h d", s=2, h=HPC),
                )

            def wo_chunk(stks, sq, chunk):
                outst = outstp.tile([128, 2, 512], F16)
                mrange = slice(chunk * 128, (chunk + 1) * 128)
                wops = psum.tile([128, 1024], F32, tag="wo", bufs=1, name="wops")
                for nh in range(2):
                    for pair in range(HPC // 2):
                        nc.tensor.matmul(
                            wops[:, nh * 512 : (nh + 1) * 512],
                            lhsT=stks[pair][:, mrange],
                            rhs=wo16[:, pair, nh * 512 : (nh + 1) * 512],
                            start=(pair == 0),
                            stop=(pair == HPC // 2 - 1),
                        )
                    nc.vector.tensor_copy(
                        outst[:, nh, :], wops[:, nh * 512 : (nh + 1) * 512]
                    )
                row0 = sq * 512 + chunk * 128
                nc.sync.dma_start(
                    out=out.ap()[row0 : row0 + 128, :],
                    in_=outst.rearrange("p a b -> p (a b)"),
                )

            def attention_block(sq, pair, extras, lag=2):
                """One (sq, pair): scores -> exp -> AV -> normalized stk tile.

                extras: list of closures emitting background PE work, spread
                evenly across the 16 skt slots (exp-paced slack filling).
                """
                cols = slice(sq * 512, (sq + 1) * 512)
                # av[:, x*512:(x+1)*512]: rows 0:64 = r bcast, 64:128 = out_h
                av = psum.tile([128, 1024], F32, tag="av", bufs=1, name="av")

                def av_mms(et, skt):
                    for x in range(2):
                        nc.tensor.matmul(
                            av[:, x * 512 : (x + 1) * 512],
                            lhsT=vh[:, skt, 2 * pair + x, :],
                            rhs=et[:, x, :],
                            start=(skt == 0),
                            stop=(skt == SKT - 1),
                        )

                pend = []
                done = 0
                for skt in range(SKT):
                    scps = psum.tile([128, 1024], F32, tag="sc", bufs=2, name="scps")
                    kcols = slice(skt * 128, (skt + 1) * 128)
                    nc.tensor.matmul(
                        scps[:, 0:512],
                        lhsT=khT[0:64, pair, kcols],
                        rhs=qhT[0:64, pair, cols],
                        start=True,
                        stop=True,
                    )
                    nc.tensor.matmul(
                        scps[:, 512:1024],
                        lhsT=khT[64:128, pair, kcols],
                        rhs=qhT[64:128, pair, cols],
                        start=True,
                        stop=True,
                    )
                    if len(pend) >= lag:
                        av_mms(*pend.pop(0))
                    et = etp.tile([128, 2, 512], F16)
                    nc.scalar.activation(
                        et.rearrange("p a b -> p (a b)"),
                        scps[:, :],
                        mybir.ActivationFunctionType.Exp,
                        scale=1.0 / np.sqrt(DK).item(),
                    )
                    pend.append((et, skt))
                    # pump extras: distribute evenly across the 16 slots
                    want = len(extras) * (skt + 1) // SKT
                    while done < want:
                        extras[done]()
                        done += 1
                while pend:
                    av_mms(*pend.pop(0))
                # normalize: read av (PSUM) only from the copy + rcp, so the
                # single av bank frees after ~2us; the muls then run on SBUF
                # operands that share start partition 0 (verifier requires
                # SBUF inputs of tensor_tensor to start on the same partition).
                avo = misc.tile([64, 1024], F32, tag="avo")
                nc.vector.tensor_copy(avo, av[64:128, :])
                rcp = misc.tile([64, 1024], F32, tag="rcp")
                nc.vector.reciprocal_approx_fast(out=rcp, in_=av[0:64, :])
                stk = stkp.tile([128, 512], F16, tag="stk")
                nc.vector.tensor_mul(stk[0:64, :], avo[:, 0:512], rcp[:, 0:512])
                nc.vector.tensor_mul(
                    stk[64:128, :], avo[:, 512:1024], rcp[:, 512:1024]
                )
                return stk

            # ---------------- emission ----------------
            # DMA priority order (sync queue): earliest-needed first.
            dma_w_chunk(wk16, wk, 0)
            ksts = [stage_tile(kre, 0, "kst")]
            dma_w_chunk(wq16, wq, 0)
            qsts = [stage_tile(qre, 0, "qst")]
            ksts.append(stage_tile(kre, 1, "kst"))
            nc.sync.dma_start(out=wv16, in_=wv.ap().rearrange("(t p) m -> p t m", p=128))
            vsts = [stage_tile(vre, 0, "vst")]
            ksts.append(stage_tile(kre, 2, "kst"))
            qsts.append(stage_tile(qre, 1, "qst"))
            vsts.append(stage_tile(vre, 1, "vst"))
            ksts.append(stage_tile(kre, 3, "kst"))
            qsts.append(stage_tile(qre, 2, "qst"))
            vsts.append(stage_tile(vre, 2, "vst"))
            qsts.append(stage_tile(qre, 3, "qst"))
            vsts.append(stage_tile(vre, 3, "vst"))
            # later-needed weights on the scalar HWDGE queue
            for m in range(1, 4):
                dma_w_chunk(wk16, wk, m, q=1)
                dma_w_chunk(wq16, wq, m, q=1)
            nc.scalar.dma_start(
                out=wo16, in_=wo.ap().rearrange("(t p) n -> p t n", p=128)
            )

            # prologue PE: khT/qhT pair 0, sb0 only — first scores ASAP
            proj_chunk(khT, wk16, ksts[0], 0, 0)
            proj_chunk(qhT, wq16, qsts[0], 0, 0)

            # extras, keyed by block index (16 blocks, pair-major)
            extras_by_block = {i: [] for i in range(16)}
            # block 0: rest of khT0 (needed by its own later skt), V
            # projection (needed by its own AVs), rest of qhT0 (needed by
            # blocks 1-3)
            eb0 = extras_by_block[0]
            eb0.append(lambda: proj_chunk(khT, wk16, ksts[1], 0, 1))
            eb0.append(lambda j=0: pv_dchunk(vsts, j))
            eb0.append(lambda: proj_chunk(khT, wk16, ksts[2], 0, 2))
            eb0.append(lambda j=1: pv_dchunk(vsts, j))
            eb0.append(lambda: proj_chunk(khT, wk16, ksts[3], 0, 3))
            eb0.append(lambda j=2: pv_dchunk(vsts, j))
            eb0.append(lambda j=3: pv_dchunk(vsts, j))
            eb0.append(lambda: proj_chunk(qhT, wq16, qsts[1], 0, 1))
            eb0.append(lambda j=4: pv_dchunk(vsts, j))
            eb0.append(lambda j=5: pv_dchunk(vsts, j))
            eb0.append(lambda: proj_chunk(qhT, wq16, qsts[2], 0, 2))
            eb0.append(lambda j=6: pv_dchunk(vsts, j))
            eb0.append(lambda j=7: pv_dchunk(vsts, j))
            eb0.append(lambda: proj_chunk(qhT, wq16, qsts[3], 0, 3))

            # deferred pair projections m=1..3: re-stage kT/qT (SBUF can't
            # hold them) and spread the 8 proj chunks over two blocks each.
            restage = {}

            def emit_restage(m):
                restage[m] = (
                    [stage_tile(kre, sb, "kst") for sb in range(SB)],
                    [stage_tile(qre, sb, "qst") for sb in range(SB)],
                )

            def defer_proj(m, b0, b1):
                for sb in range(SB):
                    extras_by_block[b0 if sb < 2 else b1].append(
                        lambda m=m, sb=sb: proj_chunk(khT, wk16, restage[m][0][sb], m, sb)
                    )
                    extras_by_block[b0 if sb < 2 else b1].append(
                        lambda m=m, sb=sb: proj_chunk(qhT, wq16, restage[m][1][sb], m, sb)
                    )

            defer_proj(1, 1, 2)
            defer_proj(2, 3, 4)
            defer_proj(3, 5, 6)

            # Wo: wo(sq) chunks fill slack in the pair-3 phase.
            # block 12=(0,3), 13=(1,3), 14=(2,3), 15=(3,3): wo(sq) pumped
            # one block after (sq,3) completes; wo(3) trails as the tail.
            stks_all = {}
            for sq in range(3):
                for chunk in range(4):
                    extras_by_block[13 + sq].append(
                        lambda sq=sq, chunk=chunk: wo_chunk(
                            [stks_all[(sq, p)] for p in range(4)], sq, chunk
                        )
                    )

            # block loop, pair-major
            bi = 0
            for pair in range(4):
                for sq in range(SB):
                    lag = 6 if bi == 0 else 2
                    stks_all[(sq, pair)] = attention_block(
                        sq, pair, extras_by_block[bi], lag=lag
                    )
                    if bi == 0:
                        emit_restage(1)
                    elif bi == 2:
                        emit_restage(2)
                    elif bi == 4:
                        emit_restage(3)
                    bi += 1
            # tail: wo(3)
            for chunk in range(4):
                wo_chunk([stks_all[(3, p)] for p in range(4)], 3, chunk)

    nc.compile()
    return nc


def _get_nc():
    if "nc" not in _CACHE:
        _CACHE["nc"] = _build()
    return _CACHE["nc"]


def build_in_maps(q, k, v, Wq, Wk, Wv, Wo):
    """Host prep: shard, cast fp16, pre-transpose activations to [D, S]."""
    q = np.asarray(q, dtype=np.float32)
    k = np.asarray(k, dtype=np.float32)
    v = np.asarray(v, dtype=np.float32)
    wq16 = np.asarray(Wq, dtype=np.float32).astype(np.float16)
    wk16 = np.asarray(Wk, dtype=np.float32).astype(np.float16)
    wv16 = np.asarray(Wv, dtype=np.float32).astype(np.float16)
    wo16 = np.asarray(Wo, dtype=np.float32).astype(np.float16)
    qT = [np.ascontiguousarray(q[b].T).astype(np.float16) for b in range(4)]
    kTt = [np.ascontiguousarray(k[b].T).astype(np.float16) for b in range(4)]
    vTt = [np.ascontiguousarray(v[b].T).astype(np.float16) for b in range(4)]
    in_maps = []
    for c in range(N_CORES):
        b, hh = c // 2, c % 2
        sl = slice(hh * HD, (hh + 1) * HD)
        in_maps.append(
            {
                "qT": qT[b],
                "kT": kTt[b],
                "vT": vTt[b],
                "wq": np.ascontiguousarray(wq16[:, sl]),
                "wk": np.ascontiguousarray(wk16[:, sl]),
                "wv": np.ascontiguousarray(wv16[:, sl]),
                "wo": np.ascontiguousarray(wo16[sl, :]),
            }
        )
    return in_maps


def kernel(q, k, v, Wq, Wk, Wv, Wo):
    nc = _get_nc()
    in_maps = build_in_maps(q, k, v, Wq, Wk, Wv, Wo)
    res = run_bass_kernel_spmd(nc, in_maps, core_ids=list(range(N_CORES)))
    outs = [res.results[c]["out"].astype(np.float32) for c in range(N_CORES)]
    return np.stack([outs[2 * b] + outs[2 * b + 1] for b in range(4)], axis=0)


# revision 11
# speedup vs baseline: 1.0874x; 1.0874x over previous
"""Multi-head attention TRN2 Bass kernel (8 NeuronCores).

Problem: B=4, S=2048, D_MODEL=1024, H=16, d_k=d_v=64 (fp32 in/out).

Sharding: core c handles batch b=c//2 and head-half hh=c%2 (8 heads).
Each core computes partial_out = softmax(qh@khT/8) @ vh @ Wo[rows of its
heads]; the host sums the two partials per batch.

The kernel is ACT(exp)-bound: 33.5M exps/core at 1 elem/cycle/lane
@1.2GHz (+284cyc per instr) = 277us ACT busy vs 273us of PE streaming.
The schedule therefore keeps the exp stream saturated end-to-end:

  - pair-major block order: attention blocks run (sq 0..3) x pair 0,
    then pair 1, ... so the projection work for pairs 1-3 (and the V
    projection) spreads into PE slack slots instead of bunching before
    the first exp.
  - prologue projects only khT/qhT (pair 0, sb0) before the first
    scores; everything else is interleaved as per-slot "extras".
  - kT/qT staging tiles are re-DMAed per projection pass (SBUF can't
    hold them across the deferred pair projections).
  - PSUM: scores 2x[128,1024] (double-buffered exp pipeline),
    av 1x[128,1024] (released early via an fp32 copy to SBUF before
    the reciprocal/muls), wo/projection chunks share 1x[128,1024].
  - scores are emitted `lag` skt-slots ahead of the matching AV matmul
    so an exp-wait never head-of-line-blocks the next scores in the
    in-order PE queue.
  - Wo for each sq runs as slack-filler extras in the pair-3 phase;
    only wo(3) trails the last block.
"""

import numpy as np

import concourse.bass as bass  # noqa: F401
import concourse.mybir as mybir
import concourse.tile as tile
from concourse import bacc
from concourse.bass_utils import run_bass_kernel_spmd

S = 2048  # sequence length
D = 1024  # d_model
HPC = 8  # heads per core
DK = 64  # head dim
HD = HPC * DK  # 512: projected width per core
N_CORES = 8

SB = S // 512  # 4 s-blocks of 512
KT = D // 128  # 8 contraction tiles for projections
SKT = S // 128  # 16 key tiles
F32 = mybir.dt.float32
F16 = mybir.dt.float16

_CACHE = {}


def _build():
    nc = bacc.Bacc("TRN2", target_bir_lowering=False, debug=False, num_devices=N_CORES)
    qT = nc.dram_tensor("qT", [D, S], F16, kind="ExternalInput")
    kT = nc.dram_tensor("kT", [D, S], F16, kind="ExternalInput")
    vT = nc.dram_tensor("vT", [D, S], F16, kind="ExternalInput")
    wq = nc.dram_tensor("wq", [D, HD], F16, kind="ExternalInput")
    wk = nc.dram_tensor("wk", [D, HD], F16, kind="ExternalInput")
    wv = nc.dram_tensor("wv", [D, HD], F16, kind="ExternalInput")
    wo = nc.dram_tensor("wo", [HD, D], F16, kind="ExternalInput")
    out = nc.dram_tensor("out", [S, D], F16, kind="ExternalOutput")

    with tile.TileContext(nc) as tc:
        with (
            tc.tile_pool(name="resident", bufs=1) as resident,
            tc.tile_pool(name="stage", bufs=2) as stage,
            tc.tile_pool(name="et", bufs=8) as etp,
            tc.tile_pool(name="misc", bufs=2) as misc,
            tc.tile_pool(name="stk", bufs=16) as stkp,
            tc.tile_pool(name="outst", bufs=2) as outstp,
            tc.tile_pool(name="psum", bufs=2, space="PSUM") as psum,
        ):
            # --- resident tiles ---
            wv16 = resident.tile([128, KT, HD], F16)
            wk16 = resident.tile([128, KT, HD], F16)
            wq16 = resident.tile([128, KT, HD], F16)
            wo16 = resident.tile([128, HD // 128, D], F16)
            qhT = resident.tile([128, HPC // 2, S], F16)  # [2-head tile, pair, Sq]
            khT = resident.tile([128, HPC // 2, S], F16)
            # AV stationary: [..., 0:64] = 1.0 (denominator), [..., 64:128] = vh
            vh = resident.tile([128, SKT, HPC, 128], F16)
            nc.vector.memset(vh[:, :, :, 0:DK], 1.0)

            kre = kT.ap().rearrange("(t p) s -> p t s", p=128)
            qre = qT.ap().rearrange("(t p) s -> p t s", p=128)
            vre = vT.ap().rearrange("(t p) s -> p t s", p=128)

            def dma_w_chunk(w16, wsrc, m, q=0):
                eng = nc.sync if q == 0 else nc.scalar
                eng.dma_start(
                    out=w16[:, :, m * 128 : (m + 1) * 128],
                    in_=wsrc.ap().rearrange("(t p) m -> p t m", p=128)[
                        :, :, m * 128 : (m + 1) * 128
                    ],
                )

            def stage_tile(src_re, sb, tag, q=0):
                st = stage.tile([128, KT, 512], F16, tag=tag, name=f"st_{tag}")
                eng = nc.sync if q == 0 else nc.scalar
                for t in range(KT):
                    eng.dma_start(out=st[:, t, :], in_=src_re[:, t, sb * 512 : (sb + 1) * 512])
                return st

            def proj_chunk(dstT, w16, st, m, sb):
                """One sb-chunk of the m-th pair projection: 8 MMs + copy."""
                ps = psum.tile([128, 1024], F32, tag="wo", bufs=1, name="pps")
                for t in range(KT):
                    nc.tensor.matmul(
                        ps[:, 0:512],
                        lhsT=w16[:, t, m * 128 : (m + 1) * 128],
                        rhs=st[:, t, :],
                        start=(t == 0),
                        stop=(t == KT - 1),
                    )
                nc.vector.tensor_copy(
                    dstT[:, m, sb * 512 : (sb + 1) * 512], ps[:, 0:512]
                )

            def proj_dchunk(dstT, w16, st0, st1, m, sb0):
                """Two sb-chunks (sb0, sb0+1) of the m-th pair projection.

                Each weight stationary is loaded once and streams both sb
                moving tiles (halves the LDWEIGHTS count), accumulating into
                the two halves of one [128,1024] PSUM tile; single copy out.
                """
                ps = psum.tile([128, 1024], F32, tag="wo", bufs=1, name="pps2")
                for t in range(KT):
                    for h, st in ((0, st0), (1, st1)):
                        nc.tensor.matmul(
                            ps[:, h * 512 : (h + 1) * 512],
                            lhsT=w16[:, t, m * 128 : (m + 1) * 128],
                            rhs=st[:, t, :],
                            start=(t == 0),
                            stop=(t == KT - 1),
                        )
                nc.vector.tensor_copy(
                    dstT[:, m, sb0 * 512 : (sb0 + 2) * 512], ps
                )

            def pv_dchunk(vsts, j):
                """V projection for skt tiles 2j, 2j+1 (all 8 heads)."""
                ps = psum.tile([128, 1024], F32, tag="wo", bufs=1, name="pvps")
                for half in range(2):
                    c = 2 * j + half
                    sb, cc = c // 4, c % 4
                    for t in range(KT):
                        nc.tensor.matmul(
                            ps[:, half * 512 : (half + 1) * 512],
                            lhsT=vsts[sb][:, t, cc * 128 : (cc + 1) * 128],
                            rhs=wv16[:, t, :],
                            start=(t == 0),
                            stop=(t == KT - 1),
                        )
                nc.vector.tensor_copy(
                    vh[:, 2 * j : 2 * j + 2, :, DK:128],
                    ps.rearrange("p (s h d) -> p s h d", s=2, h=HPC),
                )

            def wo_chunk(stks, sq, chunk, tag="wo"):
                outst = outstp.tile([128, 2, 512], F16)
                mrange = slice(chunk * 128, (chunk + 1) * 128)
                wops = psum.tile(
                    [128, 1024], F32, tag=tag, bufs=2 if tag == "sc" else 1,
                    name="wops",
                )
                # pair-outer so each stk stationary is loaded once for both
                # nh halves (halves the LDWEIGHTS count)
                for pair in range(HPC // 2):
                    for nh in range(2):
                        nc.tensor.matmul(
                            wops[:, nh * 512 : (nh + 1) * 512],
                            lhsT=stks[pair][:, mrange],
                            rhs=wo16[:, pair, nh * 512 : (nh + 1) * 512],
                            start=(pair == 0),
                            stop=(pair == HPC // 2 - 1),
                        )
                for nh in range(2):
                    nc.vector.tensor_copy(
                        outst[:, nh, :], wops[:, nh * 512 : (nh + 1) * 512]
                    )
                row0 = sq * 512 + chunk * 128
                nc.sync.dma_start(
                    out=out.ap()[row0 : row0 + 128, :],
                    in_=outst.rearrange("p a b -> p (a b)"),
                )

            def attention_block(sq, pair, extras, lag=2):
                """One (sq, pair): scores -> exp -> AV -> normalized stk tile.

                extras: list of closures emitting background PE work, spread
                evenly across the 16 skt slots (exp-paced slack filling).
                """
                cols = slice(sq * 512, (sq + 1) * 512)
                # av[:, x*512:(x+1)*512]: rows 0:64 = r bcast, 64:128 = out_h
                av = psum.tile([128, 1024], F32, tag="av", bufs=1, name="av")

                def av_mms(et, skt):
                    for x in range(2):
                        nc.tensor.matmul(
                            av[:, x * 512 : (x + 1) * 512],
                            lhsT=vh[:, skt, 2 * pair + x, :],
                            rhs=et[:, x, :],
                            start=(skt == 0),
                            stop=(skt == SKT - 1),
                        )

                pend = []
                done = 0
                for skt in range(SKT):
                    scps = psum.tile([128, 1024], F32, tag="sc", bufs=2, name="scps")
                    kcols = slice(skt * 128, (skt + 1) * 128)
                    nc.tensor.matmul(
                        scps[:, 0:512],
                        lhsT=khT[0:64, pair, kcols],
                        rhs=qhT[0:64, pair, cols],
                        start=True,
                        stop=True,
                    )
                    nc.tensor.matmul(
                        scps[:, 512:1024],
                        lhsT=khT[64:128, pair, kcols],
                        rhs=qhT[64:128, pair, cols],
                        start=True,
                        stop=True,
                    )
                    if len(pend) >= lag:
                        av_mms(*pend.pop(0))
                    et = etp.tile([128, 2, 512], F16)
                    nc.scalar.activation(
                        et.rearrange("p a b -> p (a b)"),
                        scps[:, :],
                        mybir.ActivationFunctionType.Exp,
                        scale=1.0 / np.sqrt(DK).item(),
                    )
                    pend.append((et, skt))
                    # pump extras: distribute evenly across the 16 slots
                    want = len(extras) * (skt + 1) // SKT
                    while done < want:
                        extras[done]()
                        done += 1
                while pend:
                    av_mms(*pend.pop(0))
                # normalize: read av (PSUM) only from the copy + rcp, so the
                # single av bank frees after ~2us; the muls then run on SBUF
                # operands that share start partition 0 (verifier requires
                # SBUF inputs of tensor_tensor to start on the same partition).
                avo = misc.tile([64, 1024], F32, tag="avo")
                nc.vector.tensor_copy(avo, av[64:128, :])
                rcp = misc.tile([64, 1024], F32, tag="rcp")
                nc.vector.reciprocal_approx_fast(out=rcp, in_=av[0:64, :])
                stk = stkp.tile([128, 512], F16, tag="stk")
                nc.vector.tensor_mul(stk[0:64, :], avo[:, 0:512], rcp[:, 0:512])
                nc.vector.tensor_mul(
                    stk[64:128, :], avo[:, 512:1024], rcp[:, 512:1024]
                )
                return stk

            # ---------------- emission ----------------
            # DMA priority order (sync queue): earliest-needed first.
            dma_w_chunk(wk16, wk, 0)
            ksts = [stage_tile(kre, 0, "kst")]
            dma_w_chunk(wq16, wq, 0)
            qsts = [stage_tile(qre, 0, "qst")]
            ksts.append(stage_tile(kre, 1, "kst"))
            nc.sync.dma_start(out=wv16, in_=wv.ap().rearrange("(t p) m -> p t m", p=128))
            vsts = [stage_tile(vre, 0, "vst")]
            ksts.append(stage_tile(kre, 2, "kst"))
            qsts.append(stage_tile(qre, 1, "qst"))
            vsts.append(stage_tile(vre, 1, "vst"))
            ksts.append(stage_tile(kre, 3, "kst"))
            qsts.append(stage_tile(qre, 2, "qst"))
            vsts.append(stage_tile(vre, 2, "vst"))
            qsts.append(stage_tile(qre, 3, "qst"))
            vsts.append(stage_tile(vre, 3, "vst"))
            # later-needed weights on the scalar HWDGE queue
            for m in range(1, 4):
                dma_w_chunk(wk16, wk, m, q=1)
                dma_w_chunk(wq16, wq, m, q=1)
            nc.scalar.dma_start(
                out=wo16, in_=wo.ap().rearrange("(t p) n -> p t n", p=128)
            )

            # prologue PE: khT/qhT pair 0, sb0 only — first scores ASAP
            proj_chunk(khT, wk16, ksts[0], 0, 0)
            proj_chunk(qhT, wq16, qsts[0], 0, 0)

            # extras, keyed by block index (16 blocks, pair-major).
            # hard deadlines: kh0-sb* by block 0's own scores, dpv* by block
            # 0's own AVs, qh0-sbX before block X, m=1 proj before block 4,
            # m=2 before block 8, m=3 before block 12, wo(sq) after (sq,3).
            extras_by_block = {i: [] for i in range(16)}
            eb0 = extras_by_block[0]
            eb0.append(lambda: proj_dchunk(khT, wk16, ksts[1], ksts[2], 0, 1))
            eb0.append(lambda j=0: pv_dchunk(vsts, j))
            eb0.append(lambda j=1: pv_dchunk(vsts, j))
            eb0.append(lambda: proj_chunk(khT, wk16, ksts[3], 0, 3))
            eb0.append(lambda j=2: pv_dchunk(vsts, j))
            eb0.append(lambda j=3: pv_dchunk(vsts, j))
            eb0.append(lambda j=4: pv_dchunk(vsts, j))
            eb0.append(lambda: proj_chunk(qhT, wq16, qsts[1], 0, 1))
            eb0.append(lambda j=5: pv_dchunk(vsts, j))
            eb0.append(lambda j=6: pv_dchunk(vsts, j))
            eb0.append(lambda j=7: pv_dchunk(vsts, j))
            extras_by_block[1].append(
                lambda: proj_chunk(qhT, wq16, qsts[2], 0, 2)
            )
            extras_by_block[2].append(
                lambda: proj_chunk(qhT, wq16, qsts[3], 0, 3)
            )

            # deferred pair projections m=1..3: re-stage kT/qT (SBUF can't
            # hold them); double-chunks spread over two blocks per m.
            restage = {}

            def emit_restage(m):
                restage[m] = (
                    [stage_tile(kre, sb, "kst") for sb in range(SB)],
                    [stage_tile(qre, sb, "qst") for sb in range(SB)],
                )

            def defer_proj(m, b0, b1):
                for w16, x in ((wk16, 0), (wq16, 1)):
                    dstT = khT if x == 0 else qhT
                    extras_by_block[b0].append(
                        lambda m=m, w16=w16, dstT=dstT, x=x: proj_dchunk(
                            dstT, w16, restage[m][x][0], restage[m][x][1], m, 0
                        )
                    )
                    extras_by_block[b1].append(
                        lambda m=m, w16=w16, dstT=dstT, x=x: proj_dchunk(
                            dstT, w16, restage[m][x][2], restage[m][x][3], m, 2
                        )
                    )

            defer_proj(1, 2, 3)
            defer_proj(2, 5, 6)
            defer_proj(3, 7, 8)

            # Block order: pairs 0,1 sq-major, then pairs 2,3 staggered by sq
            # so each sq's wo becomes ready progressively and spreads over
            # the last 6 blocks (2 chunks per block); wo(3) is the tail.
            ORDER = [
                (0, 0), (1, 0), (2, 0), (3, 0),
                (0, 1), (1, 1), (2, 1), (3, 1),
                (0, 2), (0, 3), (1, 2), (1, 3),
                (2, 2), (2, 3), (3, 2), (3, 3),
            ]
            stks_all = {}
            for i, (sq, nch) in enumerate([(0, 2), (0, 2), (1, 2), (1, 2), (2, 2), (2, 2)]):
                b = 10 + i
                c0 = 0 if i % 2 == 0 else 2
                for chunk in range(c0, c0 + nch):
                    extras_by_block[b].append(
                        lambda sq=sq, chunk=chunk: wo_chunk(
                            [stks_all[(sq, p)] for p in range(4)], sq, chunk
                        )
                    )

            # block loop
            for bi, (sq, pair) in enumerate(ORDER):
                lag = 4 if bi == 0 else 2
                stks_all[(sq, pair)] = attention_block(
                    sq, pair, extras_by_block[bi], lag=lag
                )
                if bi == 1:
                    emit_restage(1)
                elif bi == 4:
                    emit_restage(2)
                elif bi == 6:
                    emit_restage(3)
            # tail: wo(3) across all three free psum rings (no copy serialization)
            for chunk, tag in ((0, "wo"), (1, "sc"), (2, "av"), (3, "sc")):
                wo_chunk([stks_all[(3, p)] for p in range(4)], 3, chunk, tag=tag)

    nc.compile()
    return nc


def _get_nc():
    if "nc" not in _CACHE:
        _CACHE["nc"] = _build()
    return _CACHE["nc"]


def build_in_maps(q, k, v, Wq, Wk, Wv, Wo):
    """Host prep: shard, cast fp16, pre-transpose activations to [D, S]."""
    q = np.asarray(q, dtype=np.float32)
    k = np.asarray(k, dtype=np.float32)
    v = np.asarray(v, dtype=np.float32)
    wq16 = np.asarray(Wq, dtype=np.float32).astype(np.float16)
    wk16 = np.asarray(Wk, dtype=np.float32).astype(np.float16)
    wv16 = np.asarray(Wv, dtype=np.float32).astype(np.float16)
    wo16 = np.asarray(Wo, dtype=np.float32).astype(np.float16)
    qT = [np.ascontiguousarray(q[b].T).astype(np.float16) for b in range(4)]
    kTt = [np.ascontiguousarray(k[b].T).astype(np.float16) for b in range(4)]
    vTt = [np.ascontiguousarray(v[b].T).astype(np.float16) for b in range(4)]
    in_maps = []
    for c in range(N_CORES):
        b, hh = c // 2, c % 2
        sl = slice(hh * HD, (hh + 1) * HD)
        in_maps.append(
            {
                "qT": qT[b],
                "kT": kTt[b],
                "vT": vTt[b],
                "wq": np.ascontiguousarray(wq16[:, sl]),
                "wk": np.ascontiguousarray(wk16[:, sl]),
                "wv": np.ascontiguousarray(wv16[:, sl]),
                "wo": np.ascontiguousarray(wo16[sl, :]),
            }
        )
    return in_maps


def kernel(q, k, v, Wq, Wk, Wv, Wo):
    nc = _get_nc()
    in_maps = build_in_maps(q, k, v, Wq, Wk, Wv, Wo)
    res = run_bass_kernel_spmd(nc, in_maps, core_ids=list(range(N_CORES)))
    outs = [res.results[c]["out"].astype(np.float32) for c in range(N_CORES)]
    return np.stack([outs[2 * b] + outs[2 * b + 1] for b in range(4)], axis=0)
